# revision 22
# baseline (speedup 1.0000x reference)
"""Per-sample ResNet block (conv3x3 -> relu -> conv3x3 -> +x -> relu) on 8 trn2 cores.

Full inputs: x [16,256,64,64] f32, kernel1/kernel2 [16,256,256,3,3] f32.
Sharding: pure data parallelism, 2 samples per core.

Per-core bass/tile kernel:
  - x sample is stored in SBUF as two 128-channel chunks, zero-padded to 66x66
    so each conv tap (dy,dx) is a shifted AP slice.
  - conv = sum over (ci_chunk, tap) of matmul(lhsT=kT[ci,co], rhs=x_shift[ci,n])
    accumulated in PSUM over 18 matmuls per [128 co x 512 n] tile.
  - weights are DMA'd [co, ci*9] (contiguous) and transposed on the PE
    (out = in.T via identity) to get [ci, co] tiles.
  - residual add is one extra identity matmul into the same PSUM accumulation.
  - relu via ScalarE activation evacuates PSUM -> SBUF.
  - matmuls run as float32r (fp32 bits, single-pass PE mode: 1 cyc/row at N>=256).
"""

import numpy as np
from contextlib import ExitStack

import concourse.bass as bass
import concourse.mybir as mybir
import concourse.tile as tile
from concourse import bacc
from concourse.bass_utils import run_bass_kernel_spmd
from concourse.masks import make_identity

N_CORES = 8
B_FULL = 16
BPC = B_FULL // N_CORES  # samples per core
C = 256
H = W = 64
HP = WP = 66  # padded
P = 128
CCH = C // P  # channel chunks: 2
NT = 8        # spatial tiles (rows of 8) per image: 64 rows / 8
TR = 8        # rows per spatial tile
F32 = mybir.dt.float32
F32R = mybir.dt.float32r


def _emit_conv(nc, acc_pool, out_cb, kT, src_pad, resid_pad, mm_dtype, ident):
    """Emit one full conv over the padded source image.

    kT:      SBUF tile [128(ci), CCH(cic), CCH(coc), 9(tap), 128(co)]
    src_pad: SBUF tile [128, CCH, 66, 66] (padded input, zero borders)
    resid_pad: None or padded SBUF tile; adds resid[coc] via identity matmul
    out_cb(coc, nt, acc): consume finished PSUM accumulation [128, TR, W]
    """
    for coc in range(CCH):
        for nt in range(NT):
            r0 = nt * TR
            acc = acc_pool.tile([P, TR, W], F32, tag="acc", name="acc")
            n_mm = CCH * 9 + (1 if resid_pad is not None else 0)
            i_mm = 0
            for cic in range(CCH):
                for t in range(9):
                    dy, dx = t // 3, t % 3
                    lhsT = kT[:, cic, coc, t, :].bitcast(mm_dtype)
                    rhs = src_pad[:, cic, r0 + dy:r0 + dy + TR, dx:dx + W]
                    nc.tensor.matmul(
                        acc[:],
                        lhsT,
                        rhs.bitcast(mm_dtype),
                        start=(i_mm == 0),
                        stop=(i_mm == n_mm - 1),
                    )
                    i_mm += 1
            if resid_pad is not None:
                rhs = resid_pad[:, coc, 1 + r0:1 + r0 + TR, 1:1 + W]
                nc.tensor.matmul(
                    acc[:],
                    ident.bitcast(mm_dtype),
                    rhs.bitcast(mm_dtype),
                    start=False,
                    stop=True,
                )
            out_cb(coc, nt, acc)


def _zero_pad_borders(nc, t, mm_dtype, zrow):
    """Zero the 1-px border of a [128, CCH, HP, WP] padded tile.

    Memset can't emit float32r, so borders are DVE-copied from a zeroed
    row tile with an fp32r-tagged output view.
    """
    for c in range(CCH):
        nc.vector.tensor_copy(t[:, c, 0, :].bitcast(mm_dtype), zrow[:, :WP])
        nc.vector.tensor_copy(t[:, c, HP - 1, :].bitcast(mm_dtype), zrow[:, :WP])
        nc.vector.tensor_copy(
            t[:, c, 1:HP - 1, 0].bitcast(mm_dtype), zrow[:, :HP - 2])
        nc.vector.tensor_copy(
            t[:, c, 1:HP - 1, WP - 1].bitcast(mm_dtype), zrow[:, :HP - 2])


def build_nc(mm_dtype=F32R):
    nc = bacc.Bacc("TRN2", target_bir_lowering=False, debug=False)

    x_d = nc.dram_tensor("x", [BPC, C, H, W], F32, kind="ExternalInput")
    k1_d = nc.dram_tensor("kernel1", [BPC, C, C, 3, 3], F32, kind="ExternalInput")
    k2_d = nc.dram_tensor("kernel2", [BPC, C, C, 3, 3], F32, kind="ExternalInput")
    out_d = nc.dram_tensor("out", [BPC, C, H, W], F32, kind="ExternalOutput")

    with tile.TileContext(nc) as tc, ExitStack() as ctx:
        persist = ctx.enter_context(tc.tile_pool(name="persist", bufs=1))
        kraw_p = ctx.enter_context(tc.tile_pool(name="kraw", bufs=2))
        xs_p = ctx.enter_context(tc.tile_pool(name="xs", bufs=2))
        acc_p = ctx.enter_context(tc.tile_pool(name="acc", bufs=3, space="PSUM"))
        tr_p = ctx.enter_context(tc.tile_pool(name="tr", bufs=3, space="PSUM"))
        out_p = ctx.enter_context(tc.tile_pool(name="outs", bufs=4))

        ident = persist.tile([P, P], F32, tag="ident")
        make_identity(nc, ident)
        if mm_dtype != F32:
            # separate rounded copy: the fp32r matmul must consume data whose
            # producer instruction emits fp32r (walrus birverifier rule)
            ident_r = persist.tile([P, P], F32, tag="ident_r", name="ident_r")
            nc.vector.tensor_copy(ident_r.bitcast(mm_dtype), ident)
            ident = ident_r

        # persistent padded images + transposed weights (manually ping-ponged)
        xp = [persist.tile([P, CCH, HP, WP], F32, tag=f"xp{i}", name=f"xp{i}")
              for i in range(2)]
        hp = persist.tile([P, CCH, HP, WP], F32, tag="hp")
        k1T = persist.tile([P, CCH, CCH, 9, P], F32, tag="k1T")
        k2T = persist.tile([P, CCH, CCH, 9, P], F32, tag="k2T")
        zrow = persist.tile([P, WP], F32, tag="zrow", name="zrow")
        nc.vector.memset(zrow, 0.0)
        for t in (*xp, hp):
            _zero_pad_borders(nc, t, mm_dtype, zrow)

        def load_and_transpose(k_d, b, kT):
            # DMA [co, ci, 9] per co-chunk (contiguous per partition), then
            # PE-transpose each [co, ci] 128x128 tap block into kT[ci, co].
            for coc in range(CCH):
                kr = kraw_p.tile([P, C, 9], F32, tag="kr", name="kr")
                nc.sync.dma_start(
                    out=kr[:],
                    in_=k_d[b, coc * P:(coc + 1) * P].rearrange(
                        "co ci kh kw -> co ci (kh kw)"),
                )
                for cic in range(CCH):
                    for t in range(9):
                        ptr = tr_p.tile([P, P], F32, tag="tr", name="ptr")
                        nc.tensor.transpose(
                            ptr[:], kr[:, cic * P:(cic + 1) * P, t], ident)
                        nc.vector.tensor_copy(
                            kT[:, cic, coc, t, :].bitcast(mm_dtype), ptr[:])

        for b in range(BPC):
            x_pad = xp[b % 2]
            for c in range(CCH):
                if mm_dtype != F32:
                    # DMA to unpadded staging, then DVE pad-insert + round
                    xs = xs_p.tile([P, H, W], F32, tag="xs", name="xs")
                    nc.sync.dma_start(out=xs[:], in_=x_d[b, c * P:(c + 1) * P])
                    nc.vector.tensor_copy(
                        x_pad[:, c, 1:1 + H, 1:1 + W].bitcast(mm_dtype), xs[:])
                else:
                    nc.sync.dma_start(
                        out=x_pad[:, c, 1:1 + H, 1:1 + W],
                        in_=x_d[b, c * P:(c + 1) * P],
                    )
            load_and_transpose(k1_d, b, k1T)

            # conv1 -> relu -> hp interior
            def h_out(coc, nt, acc):
                r0 = nt * TR
                nc.scalar.activation(
                    hp[:, coc, 1 + r0:1 + r0 + TR, 1:1 + W].bitcast(mm_dtype),
                    acc[:], mybir.ActivationFunctionType.Relu)

            _emit_conv(nc, acc_p, h_out, k1T, x_pad, None, mm_dtype, ident)

            load_and_transpose(k2_d, b, k2T)

            # conv2 + residual -> relu -> out DMA
            def y_out(coc, nt, acc):
                r0 = nt * TR
                ot = out_p.tile([P, TR, W], F32, tag="ot", name="ot")
                nc.scalar.activation(
                    ot[:], acc[:], mybir.ActivationFunctionType.Relu)
                nc.sync.dma_start(
                    out=out_d[b, coc * P:(coc + 1) * P, r0:r0 + TR, :],
                    in_=ot[:],
                )

            _emit_conv(nc, acc_p, y_out, k2T, hp, x_pad, mm_dtype, ident)

    nc.compile()
    return nc


_NC_CACHE = {}


def _get_nc(mode="f32r"):
    if mode not in _NC_CACHE:
        _NC_CACHE[mode] = build_nc(F32R if mode == "f32r" else F32)
    return _NC_CACHE[mode]


def kernel(x, kernel1, kernel2, _trace=False, _mode="f32r"):
    x = np.ascontiguousarray(np.asarray(x, dtype=np.float32))
    kernel1 = np.ascontiguousarray(np.asarray(kernel1, dtype=np.float32))
    kernel2 = np.ascontiguousarray(np.asarray(kernel2, dtype=np.float32))
    nc = _get_nc(_mode)
    in_maps = [
        {
            "x": x[i * BPC:(i + 1) * BPC],
            "kernel1": kernel1[i * BPC:(i + 1) * BPC],
            "kernel2": kernel2[i * BPC:(i + 1) * BPC],
        }
        for i in range(N_CORES)
    ]
    res = run_bass_kernel_spmd(nc, in_maps, list(range(N_CORES)), trace=_trace)
    out = np.concatenate([res.results[i]["out"] for i in range(N_CORES)], axis=0)
    if _trace:
        return out, res
    return out


# revision 25
# speedup vs baseline: 1.0287x; 1.0287x over previous
"""Per-sample ResNet block (conv3x3 -> relu -> conv3x3 -> +x -> relu) on 8 trn2 cores.

Full inputs: x [16,256,64,64] f32, kernel1/kernel2 [16,256,256,3,3] f32.
Sharding: pure data parallelism, 2 samples per core.

Per-core bass/tile kernel:
  - x sample is stored in SBUF as two 128-channel chunks, zero-padded to 66x66
    so each conv tap (dy,dx) is a shifted AP slice.
  - conv = sum over (ci_chunk, tap) of matmul(lhsT=kT[ci,co], rhs=x_shift[ci,n])
    accumulated in PSUM over 18 matmuls per [128 co x 512 n] tile.
  - weights are DMA'd [co, ci*9] (contiguous) and transposed on the PE
    (out = in.T via identity) to get [ci, co] tiles.
  - residual add is one extra identity matmul into the same PSUM accumulation.
  - relu via ScalarE activation evacuates PSUM -> SBUF.
  - matmuls run as float32r (fp32 bits, single-pass PE mode: 1 cyc/row at N>=256).
"""

import numpy as np
from contextlib import ExitStack

import concourse.bass as bass
import concourse.mybir as mybir
import concourse.tile as tile
from concourse import bacc
from concourse.bass_utils import run_bass_kernel_spmd
from concourse.masks import make_identity

N_CORES = 8
B_FULL = 16
BPC = B_FULL // N_CORES  # samples per core
C = 256
H = W = 64
HP = WP = 66  # padded
P = 128
CCH = C // P  # channel chunks: 2
NT = 8        # spatial tiles (rows of 8) per image: 64 rows / 8
TR = 8        # rows per spatial tile
F32 = mybir.dt.float32
F32R = mybir.dt.float32r


def _emit_conv(nc, acc_pool, out_cb, kT, src_pad, resid_pad, mm_dtype, ident):
    """Emit one full conv over the padded source image.

    kT:      SBUF tile [128(ci), CCH(cic), CCH(coc), 9(tap), 128(co)]
    src_pad: SBUF tile [128, CCH, 66, 66] (padded input, zero borders)
    resid_pad: None or padded SBUF tile; adds resid[coc] via identity matmul
    out_cb(coc, nt, acc): consume finished PSUM accumulation [128, TR, W]
    """
    for coc in range(CCH):
        for nt in range(NT):
            r0 = nt * TR
            acc = acc_pool.tile([P, TR, W], F32, tag="acc", name="acc")
            n_mm = CCH * 9 + (1 if resid_pad is not None else 0)
            i_mm = 0
            for cic in range(CCH):
                for t in range(9):
                    dy, dx = t // 3, t % 3
                    lhsT = kT[:, cic, coc, t, :].bitcast(mm_dtype)
                    rhs = src_pad[:, cic, r0 + dy:r0 + dy + TR, dx:dx + W]
                    nc.tensor.matmul(
                        acc[:],
                        lhsT,
                        rhs.bitcast(mm_dtype),
                        start=(i_mm == 0),
                        stop=(i_mm == n_mm - 1),
                    )
                    i_mm += 1
            if resid_pad is not None:
                rhs = resid_pad[:, coc, 1 + r0:1 + r0 + TR, 1:1 + W]
                nc.tensor.matmul(
                    acc[:],
                    ident.bitcast(mm_dtype),
                    rhs.bitcast(mm_dtype),
                    start=False,
                    stop=True,
                )
            out_cb(coc, nt, acc)


def _zero_pad_borders(nc, t, mm_dtype, zrow):
    """Zero the 1-px border of a [128, CCH, HP, WP] padded tile.

    Memset can't emit float32r, so borders are DVE-copied from a zeroed
    row tile with an fp32r-tagged output view.
    """
    for c in range(CCH):
        nc.vector.tensor_copy(t[:, c, 0, :].bitcast(mm_dtype), zrow[:, :WP])
        nc.vector.tensor_copy(t[:, c, HP - 1, :].bitcast(mm_dtype), zrow[:, :WP])
        nc.vector.tensor_copy(
            t[:, c, 1:HP - 1, 0].bitcast(mm_dtype), zrow[:, :HP - 2])
        nc.vector.tensor_copy(
            t[:, c, 1:HP - 1, WP - 1].bitcast(mm_dtype), zrow[:, :HP - 2])


def build_nc(mm_dtype=F32R):
    nc = bacc.Bacc("TRN2", target_bir_lowering=False, debug=False)

    x_d = nc.dram_tensor("x", [BPC, C, H, W], F32, kind="ExternalInput")
    k1_d = nc.dram_tensor("kernel1", [BPC, C, C, 3, 3], F32, kind="ExternalInput")
    k2_d = nc.dram_tensor("kernel2", [BPC, C, C, 3, 3], F32, kind="ExternalInput")
    out_d = nc.dram_tensor("out", [BPC, C, H, W], F32, kind="ExternalOutput")

    with tile.TileContext(nc) as tc, ExitStack() as ctx:
        persist = ctx.enter_context(tc.tile_pool(name="persist", bufs=1))
        kraw_p = ctx.enter_context(tc.tile_pool(name="kraw", bufs=2))
        xs_p = ctx.enter_context(tc.tile_pool(name="xs", bufs=2))
        acc_p = ctx.enter_context(tc.tile_pool(name="acc", bufs=4, space="PSUM"))
        tr_p = ctx.enter_context(tc.tile_pool(name="tr", bufs=4, space="PSUM"))
        out_p = ctx.enter_context(tc.tile_pool(name="outs", bufs=4))

        ident = persist.tile([P, P], F32, tag="ident")
        make_identity(nc, ident)
        if mm_dtype != F32:
            # separate rounded copy: the fp32r matmul must consume data whose
            # producer instruction emits fp32r (walrus birverifier rule)
            ident_r = persist.tile([P, P], F32, tag="ident_r", name="ident_r")
            nc.vector.tensor_copy(ident_r.bitcast(mm_dtype), ident)
            ident = ident_r

        # persistent padded images + transposed weights (manually ping-ponged)
        xp = [persist.tile([P, CCH, HP, WP], F32, tag=f"xp{i}", name=f"xp{i}")
              for i in range(2)]
        hp = persist.tile([P, CCH, HP, WP], F32, tag="hp")
        k1T = persist.tile([P, CCH, CCH, 9, P], F32, tag="k1T")
        k2T = persist.tile([P, CCH, CCH, 9, P], F32, tag="k2T")
        zrow = persist.tile([P, WP], F32, tag="zrow", name="zrow")
        nc.vector.memset(zrow, 0.0)
        for t in (*xp, hp):
            _zero_pad_borders(nc, t, mm_dtype, zrow)

        def load_k_chunk(k_d, b, coc):
            kr = kraw_p.tile([P, C, 9], F32, tag="kr", name="kr")
            nc.sync.dma_start(
                out=kr[:],
                in_=k_d[b, coc * P:(coc + 1) * P].rearrange(
                    "co ci kh kw -> co ci (kh kw)"),
            )
            return kr

        def transpose_k_chunk(kr, kT, coc):
            # PE-transpose each [co, ci] 128x128 tap block into kT[ci, co].
            for cic in range(CCH):
                for t in range(9):
                    ptr = tr_p.tile([P, P], F32, tag="tr", name="ptr")
                    nc.tensor.transpose(
                        ptr[:], kr[:, cic * P:(cic + 1) * P, t], ident)
                    nc.vector.tensor_copy(
                        kT[:, cic, coc, t, :].bitcast(mm_dtype), ptr[:])

        def load_and_transpose(k_d, b, kT):
            for coc in range(CCH):
                kr = load_k_chunk(k_d, b, coc)
                transpose_k_chunk(kr, kT, coc)

        for b in range(BPC):
            x_pad = xp[b % 2]
            # Interleave k1-chunk and x-chunk DMAs so PE transposes start as
            # soon as k1 chunk 0 lands and conv1 right after x lands.
            for c in range(CCH):
                kr = load_k_chunk(k1_d, b, c)
                if mm_dtype != F32:
                    # DMA to unpadded staging, then DVE pad-insert + round
                    xs = xs_p.tile([P, H, W], F32, tag="xs", name="xs")
                    nc.sync.dma_start(out=xs[:], in_=x_d[b, c * P:(c + 1) * P])
                    nc.vector.tensor_copy(
                        x_pad[:, c, 1:1 + H, 1:1 + W].bitcast(mm_dtype), xs[:])
                else:
                    nc.sync.dma_start(
                        out=x_pad[:, c, 1:1 + H, 1:1 + W],
                        in_=x_d[b, c * P:(c + 1) * P],
                    )
                transpose_k_chunk(kr, k1T, c)

            # conv1 -> relu -> hp interior
            def h_out(coc, nt, acc):
                r0 = nt * TR
                nc.scalar.activation(
                    hp[:, coc, 1 + r0:1 + r0 + TR, 1:1 + W].bitcast(mm_dtype),
                    acc[:], mybir.ActivationFunctionType.Relu)

            _emit_conv(nc, acc_p, h_out, k1T, x_pad, None, mm_dtype, ident)

            load_and_transpose(k2_d, b, k2T)

            # conv2 + residual -> relu -> out DMA
            def y_out(coc, nt, acc):
                r0 = nt * TR
                ot = out_p.tile([P, TR, W], F32, tag="ot", name="ot")
                nc.scalar.activation(
                    ot[:], acc[:], mybir.ActivationFunctionType.Relu)
                nc.sync.dma_start(
                    out=out_d[b, coc * P:(coc + 1) * P, r0:r0 + TR, :],
                    in_=ot[:],
                )

            _emit_conv(nc, acc_p, y_out, k2T, hp, x_pad, mm_dtype, ident)

    nc.compile()
    return nc


_NC_CACHE = {}


def _get_nc(mode="f32r"):
    if mode not in _NC_CACHE:
        _NC_CACHE[mode] = build_nc(F32R if mode == "f32r" else F32)
    return _NC_CACHE[mode]


def kernel(x, kernel1, kernel2, _trace=False, _mode="f32r"):
    x = np.ascontiguousarray(np.asarray(x, dtype=np.float32))
    kernel1 = np.ascontiguousarray(np.asarray(kernel1, dtype=np.float32))
    kernel2 = np.ascontiguousarray(np.asarray(kernel2, dtype=np.float32))
    nc = _get_nc(_mode)
    in_maps = [
        {
            "x": x[i * BPC:(i + 1) * BPC],
            "kernel1": kernel1[i * BPC:(i + 1) * BPC],
            "kernel2": kernel2[i * BPC:(i + 1) * BPC],
        }
        for i in range(N_CORES)
    ]
    res = run_bass_kernel_spmd(nc, in_maps, list(range(N_CORES)), trace=_trace)
    out = np.concatenate([res.results[i]["out"] for i in range(N_CORES)], axis=0)
    if _trace:
        return out, res
    return out


# revision 26
# speedup vs baseline: 1.0793x; 1.0492x over previous
"""Per-sample ResNet block (conv3x3 -> relu -> conv3x3 -> +x -> relu) on 8 trn2 cores.

Full inputs: x [16,256,64,64] f32, kernel1/kernel2 [16,256,256,3,3] f32.
Sharding: pure data parallelism, 2 samples per core.

Per-core bass/tile kernel:
  - x sample is stored in SBUF as two 128-channel chunks, zero-padded to 66x66
    so each conv tap (dy,dx) is a shifted AP slice.
  - conv = sum over (ci_chunk, tap) of matmul(lhsT=kT[ci,co], rhs=x_shift[ci,n])
    accumulated in PSUM over 18 matmuls per [128 co x 512 n] tile.
  - weights are DMA'd [co, ci*9] (contiguous) and transposed on the PE
    (out = in.T via identity) to get [ci, co] tiles.
  - residual add is one extra identity matmul into the same PSUM accumulation.
  - relu via ScalarE activation evacuates PSUM -> SBUF.

Modes:
  f32r: fp32 storage, matmuls in float32r (single-pass PE: 1 cyc/row at N>=512).
        Producers feeding fp32r matmuls must emit fp32r (walrus rule), so x and
        the identity get DVE rounding copies and relu/weight copies write fp32r.
  bf16: bf16 storage (cast during SWDGE DMA), bf16 matmuls; fp32 PSUM.
  f32:  plain fp32 matmuls (4 cyc/row, slow reference).
"""

import numpy as np
from contextlib import ExitStack

import concourse.bass as bass
import concourse.mybir as mybir
import concourse.tile as tile
from concourse import bacc
from concourse.bass_utils import run_bass_kernel_spmd
from concourse.masks import make_identity

N_CORES = 8
B_FULL = 16
BPC = B_FULL // N_CORES  # samples per core
C = 256
H = W = 64
HP = WP = 66  # padded
P = 128
CCH = C // P  # channel chunks: 2
NT = 8        # spatial tiles (rows of 8) per image: 64 rows / 8
TR = 8        # rows per spatial tile
F32 = mybir.dt.float32
F32R = mybir.dt.float32r
BF16 = mybir.dt.bfloat16


def build_nc(mode="f32r"):
    sd = BF16 if mode == "bf16" else F32          # storage dtype
    mmd = {"f32r": F32R, "bf16": BF16, "f32": F32}[mode]  # matmul dtype

    def mm(ap):
        # view a storage AP with the matmul dtype
        return ap.bitcast(mmd) if mmd != sd else ap

    nc = bacc.Bacc("TRN2", target_bir_lowering=False, debug=False)

    x_d = nc.dram_tensor("x", [BPC, C, H, W], F32, kind="ExternalInput")
    k1_d = nc.dram_tensor("kernel1", [BPC, C, C, 3, 3], F32, kind="ExternalInput")
    k2_d = nc.dram_tensor("kernel2", [BPC, C, C, 3, 3], F32, kind="ExternalInput")
    out_d = nc.dram_tensor("out", [BPC, C, H, W], F32, kind="ExternalOutput")

    with tile.TileContext(nc) as tc, ExitStack() as ctx:
        persist = ctx.enter_context(tc.tile_pool(name="persist", bufs=1))
        kraw_p = ctx.enter_context(tc.tile_pool(name="kraw", bufs=2))
        xs_p = ctx.enter_context(tc.tile_pool(name="xs", bufs=2))
        acc_p = ctx.enter_context(tc.tile_pool(name="acc", bufs=4, space="PSUM"))
        tr_p = ctx.enter_context(tc.tile_pool(name="tr", bufs=4, space="PSUM"))
        out_p = ctx.enter_context(tc.tile_pool(name="outs", bufs=4))

        ident = persist.tile([P, P], sd, tag="ident", name="ident")
        make_identity(nc, ident)
        if mmd == F32R:
            ident_r = persist.tile([P, P], F32, tag="ident_r", name="ident_r")
            nc.vector.tensor_copy(ident_r.bitcast(F32R), ident)
            ident = ident_r

        # persistent padded images + transposed weights
        xp = [persist.tile([P, CCH, HP, WP], sd, tag=f"xp{i}", name=f"xp{i}")
              for i in range(2)]
        hp = persist.tile([P, CCH, HP, WP], sd, tag="hp", name="hp")
        k1T = persist.tile([P, CCH, CCH, 9, P], sd, tag="k1T", name="k1T")
        k2T = persist.tile([P, CCH, CCH, 9, P], sd, tag="k2T", name="k2T")

        # zero the 1-px borders of the padded tiles (via DVE copy from a zero
        # row: fp32r can't be memset directly, and the fp32r matmul requires
        # fp32r-tagged producers)
        zrow = persist.tile([P, WP], sd, tag="zrow", name="zrow")
        nc.vector.memset(zrow, 0.0)
        for t in (*xp, hp):
            for c in range(CCH):
                nc.vector.tensor_copy(mm(t[:, c, 0, :]), zrow[:, :WP])
                nc.vector.tensor_copy(mm(t[:, c, HP - 1, :]), zrow[:, :WP])
                nc.vector.tensor_copy(mm(t[:, c, 1:HP - 1, 0]), zrow[:, :HP - 2])
                nc.vector.tensor_copy(
                    mm(t[:, c, 1:HP - 1, WP - 1]), zrow[:, :HP - 2])

        def load_k_chunk(k_d, b, coc):
            kr = kraw_p.tile([P, C, 9], sd, tag="kr", name="kr")
            src = k_d[b, coc * P:(coc + 1) * P].rearrange(
                "co ci kh kw -> co ci (kh kw)")
            if sd == F32:
                nc.sync.dma_start(out=kr[:], in_=src)
            else:
                nc.gpsimd.dma_start(out=kr[:], in_=src)  # cast f32->bf16
            return kr

        def transpose_k_chunk(kr, kT, coc):
            # PE-transpose each [co, ci] 128x128 tap block into kT[ci, co]
            for cic in range(CCH):
                for t in range(9):
                    ptr = tr_p.tile([P, P], sd, tag="tr", name="ptr")
                    nc.tensor.transpose(
                        ptr[:], kr[:, cic * P:(cic + 1) * P, t], ident
                        if mmd != F32R else ident.bitcast(F32))
                    nc.vector.tensor_copy(mm(kT[:, cic, coc, t, :]), ptr[:])

        def load_x_chunk(x_pad, b, c):
            dst = x_pad[:, c, 1:1 + H, 1:1 + W]
            src = x_d[b, c * P:(c + 1) * P]
            if mmd == F32R:
                # DMA to staging, then DVE pad-insert + fp32r rounding
                xs = xs_p.tile([P, H, W], F32, tag="xs", name="xs")
                nc.sync.dma_start(out=xs[:], in_=src)
                nc.vector.tensor_copy(dst.bitcast(F32R), xs[:])
            elif sd == BF16:
                nc.gpsimd.dma_start(out=dst, in_=src)  # cast f32->bf16
            else:
                nc.sync.dma_start(out=dst, in_=src)

        def emit_conv(out_cb, kT, src_pad, resid_pad):
            for coc in range(CCH):
                for nt in range(NT):
                    r0 = nt * TR
                    acc = acc_p.tile([P, TR, W], F32, tag="acc", name="acc")
                    n_mm = CCH * 9 + (1 if resid_pad is not None else 0)
                    i_mm = 0
                    for cic in range(CCH):
                        for t in range(9):
                            dy, dx = t // 3, t % 3
                            nc.tensor.matmul(
                                acc[:],
                                mm(kT[:, cic, coc, t, :]),
                                mm(src_pad[:, cic, r0 + dy:r0 + dy + TR,
                                           dx:dx + W]),
                                start=(i_mm == 0),
                                stop=(i_mm == n_mm - 1),
                            )
                            i_mm += 1
                    if resid_pad is not None:
                        nc.tensor.matmul(
                            acc[:],
                            ident if mmd != F32R else ident.bitcast(F32R),
                            mm(resid_pad[:, coc, 1 + r0:1 + r0 + TR, 1:1 + W]),
                            start=False,
                            stop=True,
                        )
                    out_cb(coc, nt, acc)

        for b in range(BPC):
            x_pad = xp[b % 2]
            # interleave k1-chunk and x-chunk loads so PE transposes start as
            # soon as k1 chunk 0 lands and conv1 right after x lands
            for c in range(CCH):
                kr = load_k_chunk(k1_d, b, c)
                load_x_chunk(x_pad, b, c)
                transpose_k_chunk(kr, k1T, c)

            def h_out(coc, nt, acc):
                r0 = nt * TR
                nc.scalar.activation(
                    mm(hp[:, coc, 1 + r0:1 + r0 + TR, 1:1 + W]), acc[:],
                    mybir.ActivationFunctionType.Relu)

            emit_conv(h_out, k1T, x_pad, None)

            for c in range(CCH):
                kr = load_k_chunk(k2_d, b, c)
                transpose_k_chunk(kr, k2T, c)

            def y_out(coc, nt, acc):
                r0 = nt * TR
                ot = out_p.tile([P, TR, W], F32, tag="ot", name="ot")
                nc.scalar.activation(
                    ot[:], acc[:], mybir.ActivationFunctionType.Relu)
                nc.sync.dma_start(
                    out=out_d[b, coc * P:(coc + 1) * P, r0:r0 + TR, :],
                    in_=ot[:],
                )

            emit_conv(y_out, k2T, hp, x_pad)

    nc.compile()
    return nc


_NC_CACHE = {}


def _get_nc(mode="f32r"):
    if mode not in _NC_CACHE:
        _NC_CACHE[mode] = build_nc(mode)
    return _NC_CACHE[mode]


def kernel(x, kernel1, kernel2, _trace=False, _mode="f32r"):
    x = np.ascontiguousarray(np.asarray(x, dtype=np.float32))
    kernel1 = np.ascontiguousarray(np.asarray(kernel1, dtype=np.float32))
    kernel2 = np.ascontiguousarray(np.asarray(kernel2, dtype=np.float32))
    nc = _get_nc(_mode)
    in_maps = [
        {
            "x": x[i * BPC:(i + 1) * BPC],
            "kernel1": kernel1[i * BPC:(i + 1) * BPC],
            "kernel2": kernel2[i * BPC:(i + 1) * BPC],
        }
        for i in range(N_CORES)
    ]
    res = run_bass_kernel_spmd(nc, in_maps, list(range(N_CORES)), trace=_trace)
    out = np.concatenate([res.results[i]["out"] for i in range(N_CORES)], axis=0)
    if _trace:
        return out, res
    return out
